# revision 30
# baseline (speedup 1.0000x reference)
"""Batched CRF Viterbi decode on 8 TRN2 NeuronCores.

Sharding: data-parallel over batch (16 sequences per core), transitions
replicated. The sequential forward max-plus recurrence runs on-device;
backpointer reconstruction + backtrack run on host from the bit-exact
partition history.

Device computes the 48x48 live-tag block only (START never receives
probability mass after t=0; END is never a winning argmax source since
trans[END,:] = -1000). The host reconstructs part_hist columns 48/49
with the same float ordering as the reference, so the final backtrack
stays bit-exact.

Device layout (per core, BL=16 sequences, JG=8 groups x JL=6 tags):
  partitions p = jg*16 + b, tag j = jg*6 + jl
  Per step t (C in PSUM, [128, 6, 48]):
    FT-mm   (off critical path): C(t) = I128 @ FT(t), FT = trans+feats
            built on GpSimd one step ahead
    REP-mm  (on chain): C(t) += REP128 @ mfw(t) broadcast over jl; REP
            sums the 8 zero-embedded group rows -> adds part[b, i]
    reduce  (on chain): M[p, jl] = max_i C -> hist column
    mult    (on chain): mfw(t+1) = M broadcast x G (zero-embed)
"""

import numpy as np

B, S, T = 128, 512, 50
NCORES = 8
BL = B // NCORES          # 16 sequences per core
T48 = 48                  # live tags on device
JG, JL = 8, 6             # 8 groups x 6 tags = 48
NF = JL * T48             # 288 psum columns
START, END = T - 2, T - 1


def _host_prep(feats, transitions):
    """Build per-core device input arrays (all float32)."""
    f = np.ascontiguousarray(feats, dtype=np.float32)         # (B,S,T)
    tr = np.ascontiguousarray(transitions, dtype=np.float32)  # (T,T)

    k = np.arange(128)
    # transP[p=(jg,b), (jl, i)] = trans[i, jg*6+jl], i < 48
    transP = np.empty((128, JL, T48), dtype=np.float32)
    for g in range(JG):
        transP[g * BL:(g + 1) * BL] = tr[:T48, g * JL:(g + 1) * JL].T[None]
    transP = np.ascontiguousarray(transP.reshape(128, NF))

    # REP128[k, m] = 1 if k%16 == m%16  (sum over jg of zero-embedded state)
    REP128 = (k[:, None] % BL == k[None, :] % BL).astype(np.float32)
    # REPG0: group-0 rows only — selects m_col[(0, b), :] with no embed
    REPG0 = (REP128 * (k[:, None] < BL)).astype(np.float32)
    # G[p, i] = 1 if i//6 == p//16 else 0   (zero-embed mask)
    G = ((np.arange(T48)[None, :] // JL) == (k[:, None] // BL)).astype(np.float32)

    per_core = []
    for c in range(NCORES):
        fb = f[c * BL:(c + 1) * BL]                           # (16,S,T)
        # feats_arr[p=(jg,b), t*6+jl] = feats[b, t, jg*6+jl]
        fa = np.ascontiguousarray(
            fb[:, :, :T48].reshape(BL, S, JG, JL)
            .transpose(2, 0, 1, 3).reshape(128, S * JL))

        part0 = fb[:, 0, :T48] + tr[START, :T48][None, :]     # (16, 48)
        mw0 = np.repeat(part0[None, :, :], JG, axis=0).reshape(128, T48) * G
        per_core.append({
            "feats_arr": fa,
            "transP": transP,
            "I128": np.eye(128, dtype=np.float32),
            "REP128": REP128,
            "REPG0": REPG0,
            "G": G,
            "mw0": np.ascontiguousarray(mw0.astype(np.float32)),
        })
    return per_core


def build_bass(n_steps):
    import concourse.bacc as bacc
    import concourse.mybir as mybir
    import concourse.tile as tile
    from concourse.tile_rust import add_dep_helper

    f32 = mybir.dt.float32
    nc = bacc.Bacc("TRN2", target_bir_lowering=False, debug=False,
                   num_devices=NCORES)

    feats_d = nc.declare_dram_parameter("feats_arr", [128, S * JL], f32, isOutput=False)
    transP_d = nc.declare_dram_parameter("transP", [128, NF], f32, isOutput=False)
    i128_d = nc.declare_dram_parameter("I128", [128, 128], f32, isOutput=False)
    rep_d = nc.declare_dram_parameter("REP128", [128, 128], f32, isOutput=False)
    rep0_d = nc.declare_dram_parameter("REPG0", [128, 128], f32, isOutput=False)
    g_d = nc.declare_dram_parameter("G", [128, T48], f32, isOutput=False)
    mw0_d = nc.declare_dram_parameter("mw0", [128, T48], f32, isOutput=False)
    hist_d = nc.declare_dram_parameter("hist", [128, n_steps * JL], f32, isOutput=True)

    with tile.TileContext(nc) as tc:
        with (
            tc.tile_pool(name="static", bufs=1) as sp,
            tc.tile_pool(name="state", bufs=2) as st,
            tc.tile_pool(name="psum", bufs=1, space="PSUM") as pp,
        ):
            feats_sb = sp.tile([128, S * JL], f32)
            fchunk = S * JL // 4
            for ci in range(4):
                nc.sync.dma_start(
                    out=feats_sb[:, ci * fchunk:(ci + 1) * fchunk],
                    in_=feats_d[:, ci * fchunk:(ci + 1) * fchunk])
            transP_sb = sp.tile([128, NF], f32)
            nc.sync.dma_start(out=transP_sb[:, :], in_=transP_d[:, :])
            i128_sb = sp.tile([128, 128], f32)
            nc.sync.dma_start(out=i128_sb[:, :], in_=i128_d[:, :])
            rep_sb = sp.tile([128, 128], f32)
            nc.sync.dma_start(out=rep_sb[:, :], in_=rep_d[:, :])
            rep0_sb = sp.tile([128, 128], f32)
            nc.sync.dma_start(out=rep0_sb[:, :], in_=rep0_d[:, :])
            g_sb = sp.tile([128, JG, JL], f32)
            nc.sync.dma_start(out=g_sb[:, :, :], in_=g_d[:, :].rearrange(
                "p (a b) -> p a b", a=JG))

            hist_sb = sp.tile([128, n_steps * JL], f32)

            mfw = st.tile([128, T48], f32, tag="mfw")
            nc.sync.dma_start(out=mfw[:, :], in_=mw0_d[:, :])

            transP_v = transP_sb[:, :].rearrange("p (a b) -> p a b", a=JL)

            ft_tiles = {}

            def build_ft(tt):
                # FT = fl(trans + feats): statics only; built >=1 step ahead
                ft = st.tile([128, JL, T48], f32, tag="ft%d" % (tt % 2))
                ins = nc.gpsimd.tensor_tensor(
                    out=ft[:, :, :],
                    in0=transP_v[:, :, :],
                    in1=feats_sb[:, tt * JL:(tt + 1) * JL].unsqueeze(2)
                    .broadcast_to([128, JL, T48]),
                    op=mybir.AluOpType.add)
                ft_tiles[tt] = ft
                return ins

            c_tiles = {}

            def issue_ftmm(tt, after=None):
                # C(tt) = FT(tt) via identity matmul (complete group); the
                # REP matmul later accumulates part on top.
                c_ps = pp.tile([128, JL, T48], f32, tag="C%d" % (tt % 3))
                mm = nc.tensor.matmul(
                    c_ps[:, :, :], i128_sb[:, :], ft_tiles.pop(tt)[:, :, :],
                    start=True, stop=True)
                if after is not None:
                    # ordering-only edge: keep this matmul after the REP mm
                    # in the PE queue so the reduce's wait gates on the REP
                    # mm (its true dep) and FT overlaps the reduce window
                    add_dep_helper(mm.ins, after.ins, sync=False,
                                   reason="FT-mm after REP mm")
                c_tiles[tt] = c_ps
                return mm

            DUMW = 96  # warming matmul width (fills PE idle before REP mm)
            dummy_ps = None

            def issue_warm(after):
                # statics-only matmul into a scratch bank: keeps the PE
                # clock ramped through the reduce/mult window so the next
                # REP mm's first pass doesn't run at the cold rate
                nonlocal dummy_ps
                if dummy_ps is None:
                    dummy_ps = pp.tile([128, DUMW], f32, tag="D")
                mm = nc.tensor.matmul(
                    dummy_ps[:, :], i128_sb[:, :], transP_sb[:, :DUMW],
                    start=True, stop=True, skip_group_check=True)
                add_dep_helper(mm.ins, after.ins, sync=False,
                               reason="warm-mm after FT mm")
                return mm

            build_ft(1)
            build_ft(2)
            issue_ftmm(1)
            for t in range(1, n_steps + 1):
                if t + 2 <= n_steps:
                    build_ft(t + 2)

                c_ps = c_tiles.pop(t)
                if t == 1:
                    # initial state arrives 48-wide via mw0
                    repmm = nc.tensor.matmul(
                        c_ps[:, :, :], rep_sb[:, :],
                        mfw[:, :].unsqueeze(1).broadcast_to([128, JL, T48]),
                        start=False, stop=True, skip_group_check=True)
                else:
                    # group-0 source columns: read m_col(t-1) directly with
                    # group-selective weights — no embed on the chain head;
                    # the 42-col embed mult runs in this mm's shadow
                    m_prev = hist_sb[:, (t - 2) * JL: (t - 1) * JL]
                    nc.tensor.matmul(
                        c_ps[:, :, :JL], rep0_sb[:, :],
                        m_prev.unsqueeze(1).broadcast_to([128, JL, JL]),
                        start=False, stop=True, skip_group_check=True)
                    repmm = nc.tensor.matmul(
                        c_ps[:, :, JL:], rep_sb[:, :],
                        mfw[:, JL:].unsqueeze(1).broadcast_to(
                            [128, JL, T48 - JL]),
                        start=False, stop=True, skip_group_check=True)

                if t + 1 <= n_steps:
                    ftmm = issue_ftmm(t + 1, after=repmm)
                    issue_warm(after=ftmm)

                m_col = hist_sb[:, (t - 1) * JL: t * JL]
                nc.vector.tensor_reduce(
                    m_col, c_ps[:, :, :],
                    axis=mybir.AxisListType.X, op=mybir.AluOpType.max)

                if t < n_steps:
                    mfw = st.tile([128, T48], f32, tag="mfw")
                    nc.vector.tensor_tensor(
                        out=mfw[:, JL:].rearrange("p (a b) -> p a b", a=JG - 1),
                        in0=m_col.unsqueeze(1).broadcast_to([128, JG - 1, JL]),
                        in1=g_sb[:, 1:, :],
                        op=mybir.AluOpType.mult)

                # drain finished quarters of the history while computing
                if t % 128 == 0 and t < n_steps:
                    lo, hi = (t - 128) * JL, t * JL
                    nc.sync.dma_start(out=hist_d[:, lo:hi],
                                      in_=hist_sb[:, lo:hi])

            done = (n_steps // 128) * 128 * JL if n_steps >= 128 else 0
            if n_steps * JL > done:
                nc.sync.dma_start(out=hist_d[:, done:n_steps * JL],
                                  in_=hist_sb[:, done:n_steps * JL])

    nc.compile()
    return nc


def device_model(inp, n_steps):
    """Numpy model of the device kernel (for validation)."""
    fa = inp["feats_arr"]
    transP, REP128, G = inp["transP"], inp["REP128"], inp["G"]
    mfw = inp["mw0"].copy()
    hist = np.zeros((128, n_steps * JL), dtype=np.float32)
    for t in range(1, n_steps + 1):
        C = transP.reshape(128, JL, T48).copy()
        C = C + np.repeat(
            fa[:, t * JL:(t + 1) * JL], T48, axis=1).reshape(128, JL, T48)
        C = C + np.broadcast_to(
            (REP128.T @ mfw)[:, None, :], (128, JL, T48))
        M = C.max(axis=2).astype(np.float32)
        hist[:, (t - 1) * JL: t * JL] = M
        mfw = (np.broadcast_to(M[:, None, :], (128, JG, JL)).reshape(128, T48)
               * G).astype(np.float32)
    return hist


def viterbi_host(part_hist, feats, mask, transitions):
    """Backpointer reconstruction + backtrack (bit-exact vs reference)."""
    Bv = feats.shape[0]
    lengths = mask.astype(np.int64).sum(axis=1)
    last_pos = lengths - 1
    bidx = np.arange(Bv)

    last_part = part_hist[last_pos, bidx]                     # (B, T)
    last_values = last_part[:, :, None] + transitions[None]   # (B, i, j)
    pointer = np.argmax(last_values[:, :, END], axis=1).astype(np.int32)

    decode = np.zeros((S, Bv), dtype=np.int32)
    decode[S - 1] = pointer
    ptr = pointer.copy()
    trT = np.ascontiguousarray(transitions.T)                 # (j, i)
    for t in range(S - 2, -1, -1):
        sc = feats[bidx, t + 1, ptr][:, None] + trT[ptr]      # (B, i)
        cur = sc + part_hist[t]                               # (B, i)
        bp = np.argmax(cur, axis=1).astype(np.int32)
        bp = np.where(mask[:, t + 1], bp, 0)
        at_last = last_pos == t
        new_ptr = np.where(at_last, pointer, bp).astype(np.int32)
        decode[t] = new_ptr
        ptr = new_ptr
    return decode.T


def reassemble_part_hist(results, feats, transitions, ns):
    """Device hist (48 tags, ns steps) -> full (S, B, 50) part_hist,
    reconstructing the START/END columns with reference float ordering.
    Rows > ns stay zero: the backtrack masks every sequence there
    (ns = max(lengths)-1), so they never influence the output."""
    part_hist = np.zeros((S, B, T), dtype=np.float32)
    part_hist[0] = feats[:, 0, :] + transitions[START][None, :]
    for c in range(NCORES):
        hist = results[c]["hist"]                             # (128, ns*6)
        h = hist.reshape(JG, BL, ns, JL).transpose(2, 1, 0, 3)
        part_hist[1:ns + 1, c * BL:(c + 1) * BL, :T48] = \
            h.reshape(ns, BL, T48)
    # columns START/END: part[t, b, j] = max_i fl(fl(feats[b,t,j]+trans[i,j])
    #                                             + part[t-1, b, i])
    trS = transitions[:, START][None, :]                      # (1, 50)
    trE = transitions[:, END][None, :]
    fS = feats[:, :, START]                                   # (B, S)
    fE = feats[:, :, END]
    for t in range(1, ns + 1):
        prev = part_hist[t - 1]                               # (B, 50)
        part_hist[t, :, START] = ((fS[:, t][:, None] + trS) + prev).max(axis=1)
        part_hist[t, :, END] = ((fE[:, t][:, None] + trE) + prev).max(axis=1)
    return part_hist


def kernel(feats, mask, transitions):
    from concourse.bass_utils import run_bass_kernel_spmd

    feats = np.asarray(feats, dtype=np.float32)
    mask_np = np.asarray(mask).astype(bool)
    transitions = np.asarray(transitions, dtype=np.float32)

    # part_hist rows >= max(lengths) are never read unmasked by the
    # backtrack, so the device only runs max(lengths)-1 steps
    ns = max(int(mask_np.sum(1).max()) - 1, 1)

    per_core = _host_prep(feats, transitions)
    nc = build_bass(ns)
    res = run_bass_kernel_spmd(nc, per_core, core_ids=list(range(NCORES)))

    part_hist = reassemble_part_hist(res.results, feats, transitions, ns)
    return viterbi_host(part_hist, feats, mask_np, transitions).astype(np.int32)


# revision 35
# speedup vs baseline: 1.0039x; 1.0039x over previous
"""Batched CRF Viterbi decode on 8 TRN2 NeuronCores.

Sharding: data-parallel over batch (16 sequences per core), transitions
replicated. The sequential forward max-plus recurrence runs on-device;
backpointer reconstruction + backtrack run on host from the bit-exact
partition history.

Device computes the 48x48 live-tag block only (START never receives
probability mass after t=0; END is never a winning argmax source since
trans[END,:] = -1000). The host reconstructs part_hist columns 48/49
with the same float ordering as the reference, so the final backtrack
stays bit-exact.

Device layout (per core, BL=16 sequences, JG=8 groups x JL=6 tags):
  partitions p = jg*16 + b, tag j = jg*6 + jl
  Per step t (C in PSUM, [128, 6, 48]):
    FT-mm   (off critical path): C(t) = I128 @ FT(t), FT = trans+feats
            built on GpSimd one step ahead
    REP-mm  (on chain): C(t) += REP128 @ mfw(t) broadcast over jl; REP
            sums the 8 zero-embedded group rows -> adds part[b, i]
    reduce  (on chain): M[p, jl] = max_i C -> hist column
    mult    (on chain): mfw(t+1) = M broadcast x G (zero-embed)
"""

import numpy as np

B, S, T = 128, 512, 50
NCORES = 8
BL = B // NCORES          # 16 sequences per core
T48 = 48                  # live tags on device
JG, JL = 8, 6             # 8 groups x 6 tags = 48
NF = JL * T48             # 288 psum columns
START, END = T - 2, T - 1


def _host_prep(feats, transitions):
    """Build per-core device input arrays (all float32)."""
    f = np.ascontiguousarray(feats, dtype=np.float32)         # (B,S,T)
    tr = np.ascontiguousarray(transitions, dtype=np.float32)  # (T,T)

    k = np.arange(128)
    # transP[p=(jg,b), (jl, i)] = trans[i, jg*6+jl], i < 48
    transP = np.empty((128, JL, T48), dtype=np.float32)
    for g in range(JG):
        transP[g * BL:(g + 1) * BL] = tr[:T48, g * JL:(g + 1) * JL].T[None]
    transP = np.ascontiguousarray(transP.reshape(128, NF))

    # REP128[k, m] = 1 if k%16 == m%16  (sum over jg of zero-embedded state)
    REP128 = (k[:, None] % BL == k[None, :] % BL).astype(np.float32)
    # REPG0: group-0 rows only — selects m_col[(0, b), :] with no embed
    REPG0 = (REP128 * (k[:, None] < BL)).astype(np.float32)
    # G[p, i] = 1 if i//6 == p//16 else 0   (zero-embed mask)
    G = ((np.arange(T48)[None, :] // JL) == (k[:, None] // BL)).astype(np.float32)

    per_core = []
    for c in range(NCORES):
        fb = f[c * BL:(c + 1) * BL]                           # (16,S,T)
        # feats_arr[p=(jg,b), t*6+jl] = feats[b, t, jg*6+jl]
        fa = np.ascontiguousarray(
            fb[:, :, :T48].reshape(BL, S, JG, JL)
            .transpose(2, 0, 1, 3).reshape(128, S * JL))

        part0 = fb[:, 0, :T48] + tr[START, :T48][None, :]     # (16, 48)
        mw0 = np.repeat(part0[None, :, :], JG, axis=0).reshape(128, T48) * G
        per_core.append({
            "feats_arr": fa,
            "transP": transP,
            "I128": np.eye(128, dtype=np.float32),
            "REP128": REP128,
            "REPG0": REPG0,
            "G": G,
            "mw0": np.ascontiguousarray(mw0.astype(np.float32)),
        })
    return per_core


def build_bass(n_steps):
    import concourse.bacc as bacc
    import concourse.mybir as mybir
    import concourse.tile as tile
    from concourse.tile_rust import add_dep_helper

    f32 = mybir.dt.float32
    nc = bacc.Bacc("TRN2", target_bir_lowering=False, debug=False,
                   num_devices=NCORES)

    feats_d = nc.declare_dram_parameter("feats_arr", [128, S * JL], f32, isOutput=False)
    transP_d = nc.declare_dram_parameter("transP", [128, NF], f32, isOutput=False)
    i128_d = nc.declare_dram_parameter("I128", [128, 128], f32, isOutput=False)
    rep_d = nc.declare_dram_parameter("REP128", [128, 128], f32, isOutput=False)
    rep0_d = nc.declare_dram_parameter("REPG0", [128, 128], f32, isOutput=False)
    g_d = nc.declare_dram_parameter("G", [128, T48], f32, isOutput=False)
    mw0_d = nc.declare_dram_parameter("mw0", [128, T48], f32, isOutput=False)
    hist_d = nc.declare_dram_parameter("hist", [128, n_steps * JL], f32, isOutput=True)

    with tile.TileContext(nc) as tc:
        with (
            tc.tile_pool(name="static", bufs=1) as sp,
            tc.tile_pool(name="state", bufs=2) as st,
            tc.tile_pool(name="psum", bufs=1, space="PSUM") as pp,
        ):
            feats_sb = sp.tile([128, S * JL], f32)
            fchunk = S * JL // 4
            for ci in range(4):
                nc.sync.dma_start(
                    out=feats_sb[:, ci * fchunk:(ci + 1) * fchunk],
                    in_=feats_d[:, ci * fchunk:(ci + 1) * fchunk])
            transP_sb = sp.tile([128, NF], f32)
            nc.sync.dma_start(out=transP_sb[:, :], in_=transP_d[:, :])
            i128_sb = sp.tile([128, 128], f32)
            nc.sync.dma_start(out=i128_sb[:, :], in_=i128_d[:, :])
            rep_sb = sp.tile([128, 128], f32)
            nc.sync.dma_start(out=rep_sb[:, :], in_=rep_d[:, :])
            rep0_sb = sp.tile([128, 128], f32)
            nc.sync.dma_start(out=rep0_sb[:, :], in_=rep0_d[:, :])
            g_sb = sp.tile([128, JG, JL], f32)
            nc.sync.dma_start(out=g_sb[:, :, :], in_=g_d[:, :].rearrange(
                "p (a b) -> p a b", a=JG))

            hist_sb = sp.tile([128, n_steps * JL], f32)

            mfw = st.tile([128, T48], f32, tag="mfw")
            nc.sync.dma_start(out=mfw[:, :], in_=mw0_d[:, :])

            transP_v = transP_sb[:, :].rearrange("p (a b) -> p a b", a=JL)

            ft_tiles = {}

            def build_ft(tt):
                # FT = fl(trans + feats): statics only; built >=1 step ahead
                ft = st.tile([128, JL, T48], f32, tag="ft%d" % (tt % 2))
                ins = nc.gpsimd.tensor_tensor(
                    out=ft[:, :, :],
                    in0=transP_v[:, :, :],
                    in1=feats_sb[:, tt * JL:(tt + 1) * JL].unsqueeze(2)
                    .broadcast_to([128, JL, T48]),
                    op=mybir.AluOpType.add)
                ft_tiles[tt] = ft
                return ins

            c_tiles = {}

            def issue_ftmm(tt, after=None):
                # C(tt) = FT(tt) via identity matmul (complete group); the
                # REP matmul later accumulates part on top.
                c_ps = pp.tile([128, JL, T48], f32, tag="C%d" % (tt % 3))
                mm = nc.tensor.matmul(
                    c_ps[:, :, :], i128_sb[:, :], ft_tiles.pop(tt)[:, :, :],
                    start=True, stop=True)
                if after is not None:
                    # ordering-only edge: keep this matmul after the REP mm
                    # in the PE queue so the reduce's wait gates on the REP
                    # mm (its true dep) and FT overlaps the reduce window
                    add_dep_helper(mm.ins, after.ins, sync=False,
                                   reason="FT-mm after REP mm")
                c_tiles[tt] = c_ps
                return mm

            DUMW = 96  # warming matmul width (fills PE idle before REP mm)
            dummy_ps = None

            def issue_warm(after):
                # statics-only matmul into a scratch bank: keeps the PE
                # clock ramped through the reduce/mult window so the next
                # REP mm's first pass doesn't run at the cold rate
                nonlocal dummy_ps
                if dummy_ps is None:
                    dummy_ps = pp.tile([128, DUMW], f32, tag="D")
                mm = nc.tensor.matmul(
                    dummy_ps[:, :], i128_sb[:, :], transP_sb[:, :DUMW],
                    start=True, stop=True, skip_group_check=True)
                add_dep_helper(mm.ins, after.ins, sync=False,
                               reason="warm-mm after FT mm")
                return mm

            build_ft(1)
            build_ft(2)
            issue_ftmm(1)
            for t in range(1, n_steps + 1):
                if t + 2 <= n_steps:
                    build_ft(t + 2)

                c_ps = c_tiles.pop(t)
                if t == 1:
                    # initial state arrives 48-wide via mw0
                    repmm = nc.tensor.matmul(
                        c_ps[:, :, :], rep_sb[:, :],
                        mfw[:, :].unsqueeze(1).broadcast_to([128, JL, T48]),
                        start=False, stop=True, skip_group_check=True)
                else:
                    # group-0 source columns: read m_col(t-1) directly with
                    # group-selective weights — no embed on the chain head;
                    # the 42-col embed mult runs in this mm's shadow
                    m_prev = hist_sb[:, (t - 2) * JL: (t - 1) * JL]
                    nc.tensor.matmul(
                        c_ps[:, :, :JL], rep0_sb[:, :],
                        m_prev.unsqueeze(1).broadcast_to([128, JL, JL]),
                        start=False, stop=True, skip_group_check=True)
                    repmm = nc.tensor.matmul(
                        c_ps[:, :, JL:], rep_sb[:, :],
                        mfw[:, JL:].unsqueeze(1).broadcast_to(
                            [128, JL, T48 - JL]),
                        start=False, stop=True, skip_group_check=True)

                if t + 1 <= n_steps:
                    ftmm = issue_ftmm(t + 1, after=repmm)
                    issue_warm(after=ftmm)

                m_col = hist_sb[:, (t - 1) * JL: t * JL]
                nc.vector.tensor_reduce(
                    m_col, c_ps[:, :, :],
                    axis=mybir.AxisListType.X, op=mybir.AluOpType.max)

                if t < n_steps:
                    mfw = st.tile([128, T48], f32, tag="mfw")
                    nc.vector.tensor_tensor(
                        out=mfw[:, JL:].rearrange("p (a b) -> p a b", a=JG - 1),
                        in0=m_col.unsqueeze(1).broadcast_to([128, JG - 1, JL]),
                        in1=g_sb[:, 1:, :],
                        op=mybir.AluOpType.mult)

                # drain finished quarters of the history while computing
                if t % 128 == 0 and t < n_steps:
                    lo, hi = (t - 128) * JL, t * JL
                    nc.sync.dma_start(out=hist_d[:, lo:hi],
                                      in_=hist_sb[:, lo:hi])

            done = (n_steps // 128) * 128 * JL if n_steps >= 128 else 0
            if n_steps * JL > done:
                nc.sync.dma_start(out=hist_d[:, done:n_steps * JL],
                                  in_=hist_sb[:, done:n_steps * JL])

    nc.compile()
    return nc


def device_model(inp, n_steps):
    """Numpy model of the device kernel (for validation)."""
    fa = inp["feats_arr"]
    transP, REP128, G = inp["transP"], inp["REP128"], inp["G"]
    mfw = inp["mw0"].copy()
    hist = np.zeros((128, n_steps * JL), dtype=np.float32)
    for t in range(1, n_steps + 1):
        C = transP.reshape(128, JL, T48).copy()
        C = C + np.repeat(
            fa[:, t * JL:(t + 1) * JL], T48, axis=1).reshape(128, JL, T48)
        C = C + np.broadcast_to(
            (REP128.T @ mfw)[:, None, :], (128, JL, T48))
        M = C.max(axis=2).astype(np.float32)
        hist[:, (t - 1) * JL: t * JL] = M
        mfw = (np.broadcast_to(M[:, None, :], (128, JG, JL)).reshape(128, T48)
               * G).astype(np.float32)
    return hist


def viterbi_host(part_hist, feats, mask, transitions):
    """Backpointer reconstruction + backtrack (bit-exact vs reference)."""
    Bv = feats.shape[0]
    lengths = mask.astype(np.int64).sum(axis=1)
    last_pos = lengths - 1
    bidx = np.arange(Bv)

    last_part = part_hist[last_pos, bidx]                     # (B, T)
    last_values = last_part[:, :, None] + transitions[None]   # (B, i, j)
    pointer = np.argmax(last_values[:, :, END], axis=1).astype(np.int32)

    decode = np.zeros((S, Bv), dtype=np.int32)
    decode[S - 1] = pointer
    ptr = pointer.copy()
    trT = np.ascontiguousarray(transitions.T)                 # (j, i)
    for t in range(S - 2, -1, -1):
        sc = feats[bidx, t + 1, ptr][:, None] + trT[ptr]      # (B, i)
        cur = sc + part_hist[t]                               # (B, i)
        bp = np.argmax(cur, axis=1).astype(np.int32)
        bp = np.where(mask[:, t + 1], bp, 0)
        at_last = last_pos == t
        new_ptr = np.where(at_last, pointer, bp).astype(np.int32)
        decode[t] = new_ptr
        ptr = new_ptr
    return decode.T


def reassemble_part_hist(results, feats, transitions, ns):
    """Device hist (48 tags, ns steps) -> full (S, B, 50) part_hist,
    reconstructing the START/END columns with reference float ordering.
    Rows > ns stay zero: the backtrack masks every sequence there
    (ns = max(lengths)-1), so they never influence the output."""
    part_hist = np.zeros((S, B, T), dtype=np.float32)
    part_hist[0] = feats[:, 0, :] + transitions[START][None, :]
    for c in range(NCORES):
        hist = results[c]["hist"]                             # (128, ns*6)
        h = hist.reshape(JG, BL, ns, JL).transpose(2, 1, 0, 3)
        part_hist[1:ns + 1, c * BL:(c + 1) * BL, :T48] = \
            h.reshape(ns, BL, T48)
    # columns START/END: part[t, b, j] = max_i fl(fl(feats[b,t,j]+trans[i,j])
    #                                             + part[t-1, b, i])
    trS = transitions[:, START][None, :]                      # (1, 50)
    trE = transitions[:, END][None, :]
    fS = feats[:, :, START]                                   # (B, S)
    fE = feats[:, :, END]
    for t in range(1, ns + 1):
        prev = part_hist[t - 1]                               # (B, 50)
        part_hist[t, :, START] = ((fS[:, t][:, None] + trS) + prev).max(axis=1)
        part_hist[t, :, END] = ((fE[:, t][:, None] + trE) + prev).max(axis=1)
    return part_hist


def kernel(feats, mask, transitions):
    from concourse.bass_utils import run_bass_kernel_spmd

    feats = np.asarray(feats, dtype=np.float32)
    mask_np = np.asarray(mask).astype(bool)
    transitions = np.asarray(transitions, dtype=np.float32)

    # part_hist rows >= max(lengths) are never read unmasked by the
    # backtrack, so the device only runs max(lengths)-1 steps
    ns = max(int(mask_np.sum(1).max()) - 1, 1)

    per_core = _host_prep(feats, transitions)
    nc = build_bass(ns)
    res = run_bass_kernel_spmd(nc, per_core, core_ids=list(range(NCORES)))

    part_hist = reassemble_part_hist(res.results, feats, transitions, ns)
    return viterbi_host(part_hist, feats, mask_np, transitions).astype(np.int32)
